# revision 1
# baseline (speedup 1.0000x reference)
"""Trainium2 Bass kernel for the ragged 2-layer GRU problem (nn_DeepIOFeat11).

Contract: kernel(**inputs) takes FULL numpy inputs, returns FULL [N, H] f32 output.

Strategy (see NOTES.md):
- Sort sequences by length DESC, deal round-robin across 8 cores (pure data parallel).
- Per core: one mega-"chunk"; transposed layout: 21 groups of 6 hidden rows on
  partitions (126 rows + a ones-row), sequences along the free dim, column-major
  rank -> (g = r % 21, col = r // 21) so active columns form a shrinking prefix.
- Per (layer, step): 6 small matmuls (block-diag weights, K=127, M=126, N=W_t)
  compute r/z gates (x-side + h-side accumulated in PSUM) and the two n-gate
  halves; ACT does sigmoid/tanh; DVE/GPSIMD do the gate algebra; the update is
  frozen per-sequence once t >= len via z'' = max(z, [len <= t]) on the top layer.
- W_t (active column count at step t) is baked at trace time from the actual
  lengths; all engines slice [:, :W_t].
"""

import math

import numpy as np

import concourse.bacc as bacc
import concourse.bass as bass
import concourse.mybir as mybir
import concourse.tile as tile
from concourse.bass_utils import run_bass_kernel_spmd

NC = 8          # cores
G = 21          # groups per core
HP = 6          # hidden size
P = G * HP      # 126 data partitions
KP = P + 1      # +1 ones row

F32 = mybir.dt.float32
AF = mybir.ActivationFunctionType
ALU = mybir.AluOpType


def _plan(lengths):
    """Sort desc, deal round-robin. Returns per-core rank->orig index and W schedule."""
    n = lengths.shape[0]
    order = np.argsort(-lengths, kind="stable")
    percore = [order[c::NC] for c in range(NC)]          # each desc-sorted
    s = max(len(pc) for pc in percore)
    w0 = math.ceil(s / G)
    t_max = int(lengths.max())
    # W_t = max over cores of ceil(cnt(len > t) / G)
    wts = []
    for t in range(t_max):
        w = 0
        for pc in percore:
            cnt = int(np.count_nonzero(lengths[pc] > t))
            w = max(w, math.ceil(cnt / G))
        wts.append(max(w, 1))
    return percore, w0, wts


def _build_lhst(W_ih, W_hh, b_ih, b_hh, l):
    """12 block-diag lhsT matrices -> dict[(side, gate)] of [KP, P] f32."""
    out = {}
    for side in ("x", "h"):
        Wm = W_ih[l] if side == "x" else W_hh[l]          # [18, 6]
        for qi, q in enumerate(("r", "z", "n")):
            m = np.zeros((KP, P), np.float32)
            blk = Wm[qi * HP:(qi + 1) * HP, :]           # [6(out j), 6(in k)]
            for g in range(G):
                m[g * HP:(g + 1) * HP, g * HP:(g + 1) * HP] = blk.T  # [k, j]
            if side == "x":
                bias = b_ih[l][qi * HP:(qi + 1) * HP].copy()
                if q != "n":
                    bias = bias + b_hh[l][qi * HP:(qi + 1) * HP]
            else:
                bias = (b_hh[l][qi * HP:(qi + 1) * HP]
                        if q == "n" else np.zeros(HP, np.float32))
            for g in range(G):
                m[P, g * HP:(g + 1) * HP] = bias
            out[(side, q)] = m
    return out


def _build_program(t_steps, w0, wts, n_dma_slices):
    """Trace the Bass program. Shapes depend only on (t_steps, w0, wts)."""
    nc = bacc.Bacc(None, target_bir_lowering=False)
    x_dram = nc.declare_dram_parameter("x_sb", [KP, t_steps * w0], F32, isOutput=False)
    len_dram = nc.declare_dram_parameter("len_t", [P, w0], F32, isOutput=False)
    lw_dram = nc.declare_dram_parameter("w_all", [KP, 12 * P], F32, isOutput=False)
    hinit_dram = nc.declare_dram_parameter("h_init", [KP, w0], F32, isOutput=False)
    out_dram = nc.declare_dram_parameter("out", [P, w0], F32, isOutput=True)

    with tile.TileContext(nc) as tc:
        with (
            tc.tile_pool(name="persist", bufs=1) as pp,
            tc.tile_pool(name="work", bufs=3) as wp,
            tc.tile_pool(name="psum", bufs=2, space=bass.MemorySpace.PSUM) as psp,
        ):
            x_sb = pp.tile([KP, t_steps * w0], F32)
            len_sb = pp.tile([P, w0], F32)
            h = [pp.tile([KP, w0], F32, tag=f"h{i}", name=f"h{i}") for i in range(2)]
            lw_all = pp.tile([KP, 12 * P], F32)
            nc.sync.dma_start(lw_all[:], lw_dram[:])
            lw = {}
            for i, l in enumerate(range(2)):
                for j, side in enumerate(("x", "h")):
                    for k, q in enumerate(("r", "z", "n")):
                        idx = l * 6 + j * 3 + k
                        lw[(l, side, q)] = lw_all[:, idx * P:(idx + 1) * P]
            nc.sync.dma_start(len_sb[:], len_dram[:])
            # x DMA in slices so compute can start early
            sl = math.ceil(t_steps / n_dma_slices)
            for i in range(n_dma_slices):
                a, b = i * sl * w0, min(t_steps, (i + 1) * sl) * w0
                if a < b:
                    nc.sync.dma_start(x_sb[:, a:b], x_dram[:, a:b])
            for l in range(2):
                nc.sync.dma_start(h[l][:], hinit_dram[:])

            for t in range(t_steps):
                w = wts[t]
                for l in range(2):
                    rhs_x = (x_sb[:, t * w0:t * w0 + w] if l == 0
                             else h[0][:, 0:w])
                    rhs_h = h[l][:, 0:w]
                    ps = {q: psp.tile([P, w0], F32, tag=f"ps_{q}", name=f"ps_{q}")
                          for q in ("r", "z", "xn", "hn")}
                    for q in ("r", "z"):
                        nc.tensor.matmul(ps[q][:, 0:w], lw[(l, "x", q)][:],
                                         rhs_x, start=True, stop=False)
                        nc.tensor.matmul(ps[q][:, 0:w], lw[(l, "h", q)][:],
                                         rhs_h, start=False, stop=True)
                    nc.tensor.matmul(ps["xn"][:, 0:w], lw[(l, "x", "n")][:],
                                     rhs_x, start=True, stop=True)
                    nc.tensor.matmul(ps["hn"][:, 0:w], lw[(l, "h", "n")][:],
                                     rhs_h, start=True, stop=True)

                    r_sb = wp.tile([P, w0], F32, tag="r_sb")
                    z_sb = wp.tile([P, w0], F32, tag="z_sb")
                    n_sb = wp.tile([P, w0], F32, tag="n_sb")
                    t1 = wp.tile([P, w0], F32, tag="t1")
                    nc.scalar.activation(r_sb[:, 0:w], ps["r"][:, 0:w], AF.Sigmoid)
                    nc.scalar.activation(z_sb[:, 0:w], ps["z"][:, 0:w], AF.Sigmoid)
                    nc.vector.tensor_mul(t1[:, 0:w], r_sb[:, 0:w], ps["hn"][:, 0:w])
                    nc.vector.tensor_add(t1[:, 0:w], t1[:, 0:w], ps["xn"][:, 0:w])
                    nc.scalar.activation(n_sb[:, 0:w], t1[:, 0:w], AF.Tanh)
                    if l == 1:
                        m01 = wp.tile([P, w0], F32, tag="m01")
                        nc.vector.tensor_scalar(m01[:, 0:w], len_sb[:, 0:w],
                                                float(t), None, ALU.is_le)
                        nc.vector.tensor_max(z_sb[:, 0:w], z_sb[:, 0:w],
                                             m01[:, 0:w])
                    d = wp.tile([P, w0], F32, tag="d")
                    e = wp.tile([P, w0], F32, tag="e")
                    nc.gpsimd.tensor_sub(d[:, 0:w], h[l][0:P, 0:w], n_sb[:, 0:w])
                    nc.gpsimd.tensor_mul(e[:, 0:w], z_sb[:, 0:w], d[:, 0:w])
                    nc.vector.tensor_add(h[l][0:P, 0:w], n_sb[:, 0:w], e[:, 0:w])

            nc.sync.dma_start(out_dram[:], h[1][0:P, :])
    nc.compile()
    return nc


def kernel(x, lengths, W_ih, W_hh, b_ih, b_hh):
    out, _ = kernel_traced(x=x, lengths=lengths, W_ih=W_ih, W_hh=W_hh,
                           b_ih=b_ih, b_hh=b_hh, trace=False)
    return out


def kernel_traced(x, lengths, W_ih, W_hh, b_ih, b_hh, trace=False):
    x = np.ascontiguousarray(x, np.float32)
    lengths = np.ascontiguousarray(lengths)
    n, t_dim, i_dim = x.shape
    assert i_dim == HP
    percore, w0, wts = _plan(lengths)
    t_steps = len(wts)

    lhst = {}
    for l in range(2):
        for k, v in _build_lhst(np.asarray(W_ih, np.float32), np.asarray(W_hh, np.float32),
                                np.asarray(b_ih, np.float32), np.asarray(b_hh, np.float32),
                                l).items():
            lhst[(l,) + k] = v

    in_maps = []
    for c in range(NC):
        idx = percore[c]
        s = len(idx)
        # x_core grid: [KP, t_steps, w0]; row 6g+k, col c2 -> seq rank c2*G+g
        xg = np.zeros((KP, t_steps, w0), np.float32)
        xs = x[idx][:, :t_steps, :]                      # [s, t_steps, 6]
        pad = G * w0 - s
        if pad:
            xs = np.concatenate([xs, np.zeros((pad, t_steps, HP), np.float32)], 0)
        # rank r=(col*G+g) -> row block g, col
        xr = xs.reshape(w0, G, t_steps, HP)              # [col, g, t, k]
        xg[0:P] = xr.transpose(1, 3, 2, 0).reshape(P, t_steps, w0)
        xg[P] = 1.0
        lens = lengths[idx].astype(np.float32)
        if pad:
            lens = np.concatenate([lens, np.ones(pad, np.float32)])
        lg = np.repeat(lens.reshape(w0, G), HP, axis=1).T.copy()  # [P, w0] rows 6g+k
        hinit = np.zeros((KP, w0), np.float32)
        hinit[P] = 1.0
        w_all = np.zeros((KP, 12 * P), np.float32)
        for l in range(2):
            for j, side in enumerate(("x", "h")):
                for k, q in enumerate(("r", "z", "n")):
                    idx = l * 6 + j * 3 + k
                    w_all[:, idx * P:(idx + 1) * P] = lhst[(l, side, q)]
        m = {"x_sb": xg.reshape(KP, t_steps * w0), "len_t": lg, "h_init": hinit,
             "w_all": w_all}
        in_maps.append(m)

    nc = _build_program(t_steps, w0, wts, n_dma_slices=16)
    bkr = run_bass_kernel_spmd(nc, in_maps, list(range(NC)), trace=trace)
    res = bkr.results

    out = np.zeros((n, HP), np.float32)
    for c in range(NC):
        idx = percore[c]
        og = res[c]["out"]                               # [P, w0]
        # row 6g+k, col -> rank col*G+g
        vals = og.reshape(G, HP, w0).transpose(2, 0, 1).reshape(G * w0, HP)
        out[idx] = vals[:len(idx)]
    return out, bkr



# revision 5
# speedup vs baseline: 2.2955x; 2.2955x over previous
"""Trainium2 Bass kernel for the ragged 2-layer GRU problem (nn_DeepIOFeat11).

Contract: kernel(**inputs) takes FULL numpy inputs, returns FULL [N, H] f32 output.

Strategy v2 (layer-fused, bf16):
- Sort sequences by length DESC, deal round-robin across 8 cores (data parallel).
- Transposed layout: 21 groups of 6 hidden rows on partitions (126 rows + ones
  row), sequences along the free dim, rank r -> (g = r % 21, col = r // 21) so
  active columns form a shrinking prefix; per-step active width W_t baked from
  the actual lengths at trace time.
- LAYER FUSION: layer 1 lags layer 0 by one step. A fused step f runs layer-0
  step f and layer-1 step f-1 with SHARED elementwise instructions over tiles
  shaped [126, 2, w0] (regions: dim1=0 -> layer0, dim1=1 -> layer1). One
  h-update write refreshes both layers' hidden state. 129 fused steps total.
- All matmuls in bf16 (1 cycle/row vs 4 for f32): block-diag lhsT [127, 126]
  per (layer, side, gate), biases folded into a ones-row at partition 126.
- x is packed RAGGED in bf16 (per-step width w_f, offsets baked) -> ~4x less
  DMA than the padded f32 grid.
- Freeze past sequence end: a baked mask (only the <=2 partial boundary
  columns per step) is max'ed into the z-gate PSUM *pre-sigmoid*: +40 ->
  z=1, u=sigmoid(-40)=0 exactly, so h' = u*n + z*h == h. Columns beyond W_t
  are simply not touched.
- GRU algebra: h' = u*n + z*h with u = 1-z computed off the critical path.
"""

import math

import numpy as np
import ml_dtypes

import concourse.bacc as bacc
import concourse.bass as bass
import concourse.mybir as mybir
import concourse.tile as tile
from concourse.bass_utils import run_bass_kernel_spmd

NC = 8          # cores
G = 21          # groups per core
HP = 6          # hidden size
P = G * HP      # 126 data partitions
KP = P + 1      # +1 ones row

F32 = mybir.dt.float32
BF16 = mybir.dt.bfloat16
AF = mybir.ActivationFunctionType
ALU = mybir.AluOpType
NPBF = ml_dtypes.bfloat16


def _plan(lengths):
    """Sort desc, deal round-robin. Returns per-core rank->orig index and W schedule."""
    order = np.argsort(-lengths, kind="stable")
    percore = [order[c::NC] for c in range(NC)]          # each desc-sorted
    s = max(len(pc) for pc in percore)
    w0 = math.ceil(s / G)
    t_max = int(lengths.max())
    # per-step counts per core, W_t = max over cores of ceil(cnt/G)
    cnts = np.zeros((NC, t_max), np.int64)
    for c in range(NC):
        ls = lengths[percore[c]]
        for t in range(t_max):
            cnts[c, t] = int(np.count_nonzero(ls > t))
    wts = [max(1, int(np.max(np.ceil(cnts[:, t] / G)))) for t in range(t_max)]
    return percore, w0, wts, cnts


def _build_lhst(W_ih, W_hh, b_ih, b_hh, l):
    """12 block-diag lhsT matrices -> dict[(side, gate)] of [KP, P] f32."""
    out = {}
    for side in ("x", "h"):
        Wm = W_ih[l] if side == "x" else W_hh[l]          # [18, 6]
        for qi, q in enumerate(("r", "z", "n")):
            m = np.zeros((KP, P), np.float32)
            blk = Wm[qi * HP:(qi + 1) * HP, :]           # [6(out j), 6(in k)]
            for g in range(G):
                m[g * HP:(g + 1) * HP, g * HP:(g + 1) * HP] = blk.T  # [k, j]
            if side == "x":
                bias = b_ih[l][qi * HP:(qi + 1) * HP].copy()
                if q != "n":
                    bias = bias + b_hh[l][qi * HP:(qi + 1) * HP]
            else:
                bias = (b_hh[l][qi * HP:(qi + 1) * HP]
                        if q == "n" else np.zeros(HP, np.float32))
            for g in range(G):
                m[P, g * HP:(g + 1) * HP] = bias
            out[(side, q)] = m
    return out


def _fused_schedule(wts):
    """Fused step widths vf[0..T], x offsets, total x cols."""
    t_steps = len(wts)
    vf = [wts[0]] + [wts[f - 1] for f in range(1, t_steps + 1)]
    xoff = np.zeros(t_steps, np.int64)
    acc = 0
    for f in range(t_steps):
        xoff[f] = acc
        acc += vf[f]
    return vf, xoff, acc


def _mask_schedule(wts, cnts):
    """Per l1-step tt: mask region [q_tt, wts[tt]) and offsets into M."""
    t_steps = len(wts)
    qs, ms, moff = [], [], []
    acc = 0
    for tt in range(t_steps):
        q = int(min(cnts[c, tt] // G for c in range(NC)))
        m = wts[tt] - q
        qs.append(q)
        ms.append(m)
        moff.append(acc)
        acc += m
    return qs, ms, moff, acc


def _build_program(t_steps, w0, wts, vf, xoff, xcols, qs, ms, moff, mcols,
                   n_dma_slices):
    nc = bacc.Bacc(None, target_bir_lowering=False)
    x_dram = nc.declare_dram_parameter("x_sb", [KP, xcols], BF16, isOutput=False)
    lw_dram = nc.declare_dram_parameter("w_all", [KP, 12 * P], BF16, isOutput=False)
    m_dram = nc.declare_dram_parameter("mask_t", [P, max(mcols, 1)], F32,
                                       isOutput=False)
    ones_dram = nc.declare_dram_parameter("ones_t", [1, 2 * w0], BF16,
                                          isOutput=False)
    out_dram = nc.declare_dram_parameter("out", [P, w0], BF16, isOutput=True)

    with tile.TileContext(nc) as tc:
        with (
            tc.tile_pool(name="persist", bufs=1) as pp,
            tc.tile_pool(name="work", bufs=3) as wp,
            tc.tile_pool(name="psum", bufs=2, space=bass.MemorySpace.PSUM) as psp,
        ):
            x_sb = pp.tile([KP, xcols], BF16)
            m_sb = pp.tile([P, max(mcols, 1)], F32)
            h01 = pp.tile([KP, 2, w0], BF16, tag="h01", name="h01")
            lw_all = pp.tile([KP, 12 * P], BF16)
            nc.sync.dma_start(lw_all[:], lw_dram[:])
            nc.sync.dma_start(m_sb[:], m_dram[:])
            lw = {}
            for l in range(2):
                for j, side in enumerate(("x", "h")):
                    for k, q in enumerate(("r", "z", "n")):
                        idx = l * 6 + j * 3 + k
                        lw[(l, side, q)] = lw_all[:, idx * P:(idx + 1) * P]
            # x DMA in slices so compute can start early
            sl = math.ceil(xcols / n_dma_slices)
            for i in range(n_dma_slices):
                a, b = i * sl, min(xcols, (i + 1) * sl)
                if a < b:
                    nc.sync.dma_start(x_sb[:, a:b], x_dram[:, a:b])
            nc.gpsimd.memset(h01[0:P, :, :], 0.0)
            # ones row sits at partition 126; engine ops need quarter-aligned
            # partition starts, so fill it via DMA instead of memset
            nc.sync.dma_start(h01[P:KP, :, :], ones_dram[:])

            for f in range(t_steps + 1):
                w = vf[f]
                l0 = f < t_steps
                l1 = f >= 1
                ra, rb = (0, 2) if (l0 and l1) else ((0, 1) if l0 else (1, 2))
                ps_r = psp.tile([P, 2, w0], F32, tag="ps_r", name="ps_r")
                ps_z = psp.tile([P, 2, w0], F32, tag="ps_z", name="ps_z")
                ps_xn = psp.tile([P, 2, w0], F32, tag="ps_xn", name="ps_xn")
                ps_hn = psp.tile([P, 2, w0], F32, tag="ps_hn", name="ps_hn")

                rhs_x0 = x_sb[:, xoff[f]:xoff[f] + w] if l0 else None
                rhs_h0 = h01[0:KP, 0, 0:w]
                rhs_h1 = h01[0:KP, 1, 0:w]

                # r gate matmuls first (sigmoid(r) gates the n-chain), then hn.
                if l0:
                    nc.tensor.matmul(ps_r[0:P, 0, 0:w], lw[(0, "x", "r")][:],
                                     rhs_x0, start=True, stop=False)
                    nc.tensor.matmul(ps_r[0:P, 0, 0:w], lw[(0, "h", "r")][:],
                                     rhs_h0, start=False, stop=True)
                if l1:
                    nc.tensor.matmul(ps_r[0:P, 1, 0:w], lw[(1, "x", "r")][:],
                                     rhs_h0, start=True, stop=False)
                    nc.tensor.matmul(ps_r[0:P, 1, 0:w], lw[(1, "h", "r")][:],
                                     rhs_h1, start=False, stop=True)
                if l0:
                    nc.tensor.matmul(ps_hn[0:P, 0, 0:w], lw[(0, "h", "n")][:],
                                     rhs_h0, start=True, stop=True)
                if l1:
                    nc.tensor.matmul(ps_hn[0:P, 1, 0:w], lw[(1, "h", "n")][:],
                                     rhs_h1, start=True, stop=True)
                if l0:
                    nc.tensor.matmul(ps_z[0:P, 0, 0:w], lw[(0, "x", "z")][:],
                                     rhs_x0, start=True, stop=False)
                    nc.tensor.matmul(ps_z[0:P, 0, 0:w], lw[(0, "h", "z")][:],
                                     rhs_h0, start=False, stop=True)
                if l1:
                    nc.tensor.matmul(ps_z[0:P, 1, 0:w], lw[(1, "x", "z")][:],
                                     rhs_h0, start=True, stop=False)
                    nc.tensor.matmul(ps_z[0:P, 1, 0:w], lw[(1, "h", "z")][:],
                                     rhs_h1, start=False, stop=True)
                if l0:
                    nc.tensor.matmul(ps_xn[0:P, 0, 0:w], lw[(0, "x", "n")][:],
                                     rhs_x0, start=True, stop=True)
                if l1:
                    nc.tensor.matmul(ps_xn[0:P, 1, 0:w], lw[(1, "x", "n")][:],
                                     rhs_h0, start=True, stop=True)

                # freeze mask into l1's z PSUM pre-sigmoid (z->1, u->0 exactly)
                if l1:
                    tt = f - 1
                    q, m, mo = qs[tt], ms[tt], moff[tt]
                    if m > 0:
                        nc.vector.tensor_max(ps_z[0:P, 1, q:q + m],
                                             ps_z[0:P, 1, q:q + m],
                                             m_sb[:, mo:mo + m])

                r_sb = wp.tile([P, 2, w0], BF16, tag="r_sb")
                z_sb = wp.tile([P, 2, w0], BF16, tag="z_sb")
                n_sb = wp.tile([P, 2, w0], BF16, tag="n_sb")
                u_sb = wp.tile([P, 2, w0], BF16, tag="u_sb")
                t1 = wp.tile([P, 2, w0], F32, tag="t1")
                t2 = wp.tile([P, 2, w0], F32, tag="t2")
                zh = wp.tile([P, 2, w0], BF16, tag="zh")
                un = wp.tile([P, 2, w0], BF16, tag="un")

                nc.scalar.activation(r_sb[0:P, ra:rb, 0:w], ps_r[0:P, ra:rb, 0:w],
                                     AF.Sigmoid)
                nc.scalar.activation(z_sb[0:P, ra:rb, 0:w], ps_z[0:P, ra:rb, 0:w],
                                     AF.Sigmoid)
                # n-chain (critical path): t1 = r*hn, t2 = t1+xn, n = tanh(t2)
                nc.vector.tensor_mul(t1[0:P, ra:rb, 0:w], r_sb[0:P, ra:rb, 0:w],
                                     ps_hn[0:P, ra:rb, 0:w])
                nc.vector.tensor_add(t2[0:P, ra:rb, 0:w], t1[0:P, ra:rb, 0:w],
                                     ps_xn[0:P, ra:rb, 0:w])
                nc.scalar.activation(n_sb[0:P, ra:rb, 0:w], t2[0:P, ra:rb, 0:w],
                                     AF.Tanh)
                # off-path: u = 1-z, zh = z*h
                nc.gpsimd.tensor_scalar(u_sb[0:P, ra:rb, 0:w],
                                        z_sb[0:P, ra:rb, 0:w],
                                        -1.0, 1.0, ALU.mult, ALU.add)
                nc.gpsimd.tensor_mul(zh[0:P, ra:rb, 0:w], z_sb[0:P, ra:rb, 0:w],
                                     h01[0:P, ra:rb, 0:w])
                # h' = u*n + z*h  (one write updates both layers)
                nc.vector.tensor_mul(un[0:P, ra:rb, 0:w], u_sb[0:P, ra:rb, 0:w],
                                     n_sb[0:P, ra:rb, 0:w])
                nc.gpsimd.tensor_add(h01[0:P, ra:rb, 0:w], un[0:P, ra:rb, 0:w],
                                     zh[0:P, ra:rb, 0:w])

            nc.sync.dma_start(out_dram[:], h01[0:P, 1, :])
    nc.compile()
    return nc


def kernel(x, lengths, W_ih, W_hh, b_ih, b_hh):
    out, _ = kernel_traced(x=x, lengths=lengths, W_ih=W_ih, W_hh=W_hh,
                           b_ih=b_ih, b_hh=b_hh, trace=False)
    return out


def kernel_traced(x, lengths, W_ih, W_hh, b_ih, b_hh, trace=False):
    x = np.ascontiguousarray(x, np.float32)
    lengths = np.ascontiguousarray(lengths)
    n, t_dim, i_dim = x.shape
    assert i_dim == HP
    percore, w0, wts, cnts = _plan(lengths)
    t_steps = len(wts)
    vf, xoff, xcols = _fused_schedule(wts)
    qs, ms, moff, mcols = _mask_schedule(wts, cnts)

    lhst = {}
    for l in range(2):
        for k, v in _build_lhst(np.asarray(W_ih, np.float32),
                                np.asarray(W_hh, np.float32),
                                np.asarray(b_ih, np.float32),
                                np.asarray(b_hh, np.float32), l).items():
            lhst[(l,) + k] = v
    w_all = np.zeros((KP, 12 * P), np.float32)
    for l in range(2):
        for j, side in enumerate(("x", "h")):
            for k, q in enumerate(("r", "z", "n")):
                idx = l * 6 + j * 3 + k
                w_all[:, idx * P:(idx + 1) * P] = lhst[(l, side, q)]
    w_all = w_all.astype(NPBF)

    in_maps = []
    for c in range(NC):
        idx = percore[c]
        s = len(idx)
        pad = G * w0 - s
        xs = x[idx][:, :t_steps, :]                      # [s, t_steps, 6]
        if pad:
            xs = np.concatenate([xs, np.zeros((pad, t_steps, HP), np.float32)], 0)
        xg = np.empty((KP, xcols), np.float32)
        xg[P, :] = 1.0
        for f in range(t_steps):
            v = vf[f]
            blk = xs[:G * v, f, :].reshape(v, G, HP)     # [col, g, k]
            xg[0:P, xoff[f]:xoff[f] + v] = blk.transpose(1, 2, 0).reshape(P, v)
        # masks: +40 -> frozen (z=1,u=0), -1e30 -> active (no-op under max)
        mg = np.full((P, max(mcols, 1)), -1e30, np.float32)
        for tt in range(t_steps):
            q, m, mo = qs[tt], ms[tt], moff[tt]
            if m <= 0:
                continue
            cnt = int(cnts[c, tt])
            qc, kc = cnt // G, cnt % G
            for j in range(q, wts[tt]):
                col = mg[:, mo + (j - q)]
                if j > qc or (j == qc and kc == 0):
                    col[:] = 40.0
                elif j == qc:
                    col[kc * HP:] = 40.0
        in_maps.append({"x_sb": xg.astype(NPBF), "w_all": w_all, "mask_t": mg,
                        "ones_t": np.ones((1, 2 * w0), NPBF)})

    nc = _build_program(t_steps, w0, wts, vf, xoff, xcols, qs, ms, moff, mcols,
                        n_dma_slices=10)
    bkr = run_bass_kernel_spmd(nc, in_maps, list(range(NC)), trace=trace)
    res = bkr.results

    out = np.zeros((n, HP), np.float32)
    for c in range(NC):
        idx = percore[c]
        og = np.asarray(res[c]["out"], dtype=np.float32)  # [P, w0]
        vals = og.reshape(G, HP, w0).transpose(2, 0, 1).reshape(G * w0, HP)
        out[idx] = vals[:len(idx)]
    return out, bkr


# revision 6
# speedup vs baseline: 2.8093x; 1.2238x over previous
"""Trainium2 Bass kernel for the ragged 2-layer GRU problem (nn_DeepIOFeat11).

Contract: kernel(**inputs) takes FULL numpy inputs, returns FULL [N, H] f32 output.

Strategy v3 (per-layer staggered chains, bf16):
- Sort sequences by length DESC, deal round-robin across 8 cores (data parallel).
- Transposed layout: 21 groups of 6 hidden rows on partitions (126 rows + ones
  row), sequences along the free dim, rank r -> (g = r % 21, col = r // 21) so
  active columns form a shrinking prefix; per-step active width baked from the
  actual lengths at trace time.
- Layer 1 lags layer 0 by one step. Each layer runs its own dependency chain
  (own PSUM tiles, own h tile) so the two chains pipeline through the engines;
  l0's recurrence is not serialized behind l1's ops. 129 fused steps.
- All matmuls bf16 (1 cycle/row), biases folded into a ones-row at partition
  126 of the rhs. x packed RAGGED in bf16 (per-step width, offsets baked).
- Freeze past sequence end: baked mask (only the <=2 boundary columns/step)
  max'ed into the z-gate PSUM pre-sigmoid: +40 -> z=1 and u=1-z=0 exactly, so
  h' = u*n + z*h == h. Columns beyond the active width are never touched.
- GRU algebra: h' = u*n + z*h with u = 1-z and z*h computed off the critical
  path; critical chain is MM -> sig(r) -> t1 -> t2 -> tanh -> un -> h'.
"""

import math

import numpy as np
import ml_dtypes

import concourse.bacc as bacc
import concourse.bass as bass
import concourse.mybir as mybir
import concourse.tile as tile
from concourse.bass_utils import run_bass_kernel_spmd

NC = 8          # cores
G = 21          # groups per core
HP = 6          # hidden size
P = G * HP      # 126 data partitions
KP = P + 1      # +1 ones row

F32 = mybir.dt.float32
BF16 = mybir.dt.bfloat16
AF = mybir.ActivationFunctionType
ALU = mybir.AluOpType
NPBF = ml_dtypes.bfloat16


def _plan(lengths):
    """Sort desc, deal round-robin. Returns per-core rank->orig index and W schedule."""
    order = np.argsort(-lengths, kind="stable")
    percore = [order[c::NC] for c in range(NC)]          # each desc-sorted
    s = max(len(pc) for pc in percore)
    w0 = math.ceil(s / G)
    t_max = int(lengths.max())
    cnts = np.zeros((NC, t_max), np.int64)
    for c in range(NC):
        ls = lengths[percore[c]]
        for t in range(t_max):
            cnts[c, t] = int(np.count_nonzero(ls > t))
    wts = [max(1, int(np.max(np.ceil(cnts[:, t] / G)))) for t in range(t_max)]
    return percore, w0, wts, cnts


def _build_lhst(W_ih, W_hh, b_ih, b_hh, l):
    """12 block-diag lhsT matrices -> dict[(side, gate)] of [KP, P] f32."""
    out = {}
    for side in ("x", "h"):
        Wm = W_ih[l] if side == "x" else W_hh[l]          # [18, 6]
        for qi, q in enumerate(("r", "z", "n")):
            m = np.zeros((KP, P), np.float32)
            blk = Wm[qi * HP:(qi + 1) * HP, :]           # [6(out j), 6(in k)]
            for g in range(G):
                m[g * HP:(g + 1) * HP, g * HP:(g + 1) * HP] = blk.T  # [k, j]
            if side == "x":
                bias = b_ih[l][qi * HP:(qi + 1) * HP].copy()
                if q != "n":
                    bias = bias + b_hh[l][qi * HP:(qi + 1) * HP]
            else:
                bias = (b_hh[l][qi * HP:(qi + 1) * HP]
                        if q == "n" else np.zeros(HP, np.float32))
            for g in range(G):
                m[P, g * HP:(g + 1) * HP] = bias
            out[(side, q)] = m
    return out


def _fused_schedule(wts):
    """Fused step widths vf[0..T], x offsets, total x cols."""
    t_steps = len(wts)
    vf = [wts[0]] + [wts[f - 1] for f in range(1, t_steps + 1)]
    xoff = np.zeros(t_steps, np.int64)
    acc = 0
    for f in range(t_steps):
        xoff[f] = acc
        acc += vf[f]
    return vf, xoff, acc


def _mask_schedule(wts, cnts):
    """Per l1-step tt: mask region [q_tt, wts[tt]) and offsets into M."""
    t_steps = len(wts)
    qs, ms, moff = [], [], []
    acc = 0
    for tt in range(t_steps):
        q = int(min(cnts[c, tt] // G for c in range(NC)))
        m = wts[tt] - q
        qs.append(q)
        ms.append(m)
        moff.append(acc)
        acc += m
    return qs, ms, moff, acc


def _build_program(t_steps, w0, wts, vf, xoff, xcols, qs, ms, moff, mcols,
                   n_dma_slices):
    nc = bacc.Bacc(None, target_bir_lowering=False)
    x_dram = nc.declare_dram_parameter("x_sb", [KP, xcols], BF16, isOutput=False)
    lw_dram = nc.declare_dram_parameter("w_all", [KP, 12 * P], BF16, isOutput=False)
    m_dram = nc.declare_dram_parameter("mask_t", [P, max(mcols, 1)], F32,
                                       isOutput=False)
    ones_dram = nc.declare_dram_parameter("ones_t", [1, w0], BF16,
                                          isOutput=False)
    out_dram = nc.declare_dram_parameter("out", [P, w0], BF16, isOutput=True)

    with tile.TileContext(nc) as tc:
        with (
            tc.tile_pool(name="persist", bufs=1) as pp,
            tc.tile_pool(name="work", bufs=2) as wp,
            tc.tile_pool(name="psum", bufs=1, space=bass.MemorySpace.PSUM) as psp,
        ):
            x_sb = pp.tile([KP, xcols], BF16)
            m_sb = pp.tile([P, max(mcols, 1)], F32)
            h = [pp.tile([KP, w0], BF16, tag=f"h{i}", name=f"h{i}")
                 for i in range(2)]
            lw_all = pp.tile([KP, 12 * P], BF16)
            nc.sync.dma_start(lw_all[:], lw_dram[:])
            nc.sync.dma_start(m_sb[:], m_dram[:])
            lw = {}
            for l in range(2):
                for j, side in enumerate(("x", "h")):
                    for k, q in enumerate(("r", "z", "n")):
                        idx = l * 6 + j * 3 + k
                        lw[(l, side, q)] = lw_all[:, idx * P:(idx + 1) * P]
            sl = math.ceil(xcols / n_dma_slices)
            for i in range(n_dma_slices):
                a, b = i * sl, min(xcols, (i + 1) * sl)
                if a < b:
                    nc.sync.dma_start(x_sb[:, a:b], x_dram[:, a:b])
            for l in range(2):
                nc.gpsimd.memset(h[l][0:P, :], 0.0)
                # ones row at partition 126: engine ops need quarter-aligned
                # partition starts, so fill via DMA
                nc.sync.dma_start(h[l][P:KP, :], ones_dram[:])

            # persistent per-layer PSUM tiles (8 banks exactly)
            ps = {(l, g): psp.tile([P, w0], F32, tag=f"ps_{g}{l}",
                                   name=f"ps_{g}{l}")
                  for l in range(2) for g in ("r", "z", "xn", "hn")}

            for f in range(t_steps + 1):
                w = vf[f]
                layers = ([0] if f == 0 else
                          [1] if f == t_steps else [0, 1])
                # matmuls: r + hn first (they gate the chain), then xn, z
                for l in layers:
                    rhs_x = (x_sb[:, xoff[f]:xoff[f] + w] if l == 0
                             else h[0][0:KP, 0:w])
                    rhs_h = h[l][0:KP, 0:w]
                    nc.tensor.matmul(ps[(l, "r")][0:P, 0:w], lw[(l, "x", "r")][:],
                                     rhs_x, start=True, stop=False)
                    nc.tensor.matmul(ps[(l, "r")][0:P, 0:w], lw[(l, "h", "r")][:],
                                     rhs_h, start=False, stop=True)
                    nc.tensor.matmul(ps[(l, "hn")][0:P, 0:w], lw[(l, "h", "n")][:],
                                     rhs_h, start=True, stop=True)
                for l in layers:
                    rhs_x = (x_sb[:, xoff[f]:xoff[f] + w] if l == 0
                             else h[0][0:KP, 0:w])
                    rhs_h = h[l][0:KP, 0:w]
                    nc.tensor.matmul(ps[(l, "xn")][0:P, 0:w], lw[(l, "x", "n")][:],
                                     rhs_x, start=True, stop=True)
                    nc.tensor.matmul(ps[(l, "z")][0:P, 0:w], lw[(l, "x", "z")][:],
                                     rhs_x, start=True, stop=False)
                    nc.tensor.matmul(ps[(l, "z")][0:P, 0:w], lw[(l, "h", "z")][:],
                                     rhs_h, start=False, stop=True)

                for l in layers:
                    if l == 1:
                        tt = f - 1
                        q, m, mo = qs[tt], ms[tt], moff[tt]
                        if m > 0:
                            nc.vector.tensor_max(ps[(1, "z")][0:P, q:q + m],
                                                 ps[(1, "z")][0:P, q:q + m],
                                                 m_sb[:, mo:mo + m])
                    r_sb = wp.tile([P, w0], BF16, tag=f"r{l}")
                    z_sb = wp.tile([P, w0], BF16, tag=f"z{l}")
                    n_sb = wp.tile([P, w0], BF16, tag=f"n{l}")
                    u_sb = wp.tile([P, w0], BF16, tag=f"u{l}")
                    t1 = wp.tile([P, w0], F32, tag=f"t1{l}")
                    t2 = wp.tile([P, w0], F32, tag=f"t2{l}")
                    zh = wp.tile([P, w0], BF16, tag=f"zh{l}")
                    un = wp.tile([P, w0], BF16, tag=f"un{l}")

                    nc.scalar.activation(r_sb[0:P, 0:w], ps[(l, "r")][0:P, 0:w],
                                         AF.Sigmoid)
                    # critical chain: t1 = r*hn, t2 = t1+xn, n = tanh(t2)
                    nc.vector.tensor_mul(t1[0:P, 0:w], r_sb[0:P, 0:w],
                                         ps[(l, "hn")][0:P, 0:w])
                    nc.vector.tensor_add(t2[0:P, 0:w], t1[0:P, 0:w],
                                         ps[(l, "xn")][0:P, 0:w])
                    nc.scalar.activation(n_sb[0:P, 0:w], t2[0:P, 0:w], AF.Tanh)
                    nc.scalar.activation(z_sb[0:P, 0:w], ps[(l, "z")][0:P, 0:w],
                                         AF.Sigmoid)
                    # off-path: u = 1-z, zh = z*h (old h)
                    nc.gpsimd.tensor_scalar(u_sb[0:P, 0:w], z_sb[0:P, 0:w],
                                            -1.0, 1.0, ALU.mult, ALU.add)
                    nc.gpsimd.tensor_mul(zh[0:P, 0:w], z_sb[0:P, 0:w],
                                         h[l][0:P, 0:w])
                    # h' = u*n + z*h  (un, h' back-to-back on Vector)
                    nc.vector.tensor_mul(un[0:P, 0:w], u_sb[0:P, 0:w],
                                         n_sb[0:P, 0:w])
                    nc.vector.tensor_add(h[l][0:P, 0:w], un[0:P, 0:w],
                                         zh[0:P, 0:w])

            nc.sync.dma_start(out_dram[:], h[1][0:P, :])
    nc.compile()
    return nc


def kernel(x, lengths, W_ih, W_hh, b_ih, b_hh):
    out, _ = kernel_traced(x=x, lengths=lengths, W_ih=W_ih, W_hh=W_hh,
                           b_ih=b_ih, b_hh=b_hh, trace=False)
    return out


def kernel_traced(x, lengths, W_ih, W_hh, b_ih, b_hh, trace=False):
    x = np.ascontiguousarray(x, np.float32)
    lengths = np.ascontiguousarray(lengths)
    n, t_dim, i_dim = x.shape
    assert i_dim == HP
    percore, w0, wts, cnts = _plan(lengths)
    t_steps = len(wts)
    vf, xoff, xcols = _fused_schedule(wts)
    qs, ms, moff, mcols = _mask_schedule(wts, cnts)

    lhst = {}
    for l in range(2):
        for k, v in _build_lhst(np.asarray(W_ih, np.float32),
                                np.asarray(W_hh, np.float32),
                                np.asarray(b_ih, np.float32),
                                np.asarray(b_hh, np.float32), l).items():
            lhst[(l,) + k] = v
    w_all = np.zeros((KP, 12 * P), np.float32)
    for l in range(2):
        for j, side in enumerate(("x", "h")):
            for k, q in enumerate(("r", "z", "n")):
                idx = l * 6 + j * 3 + k
                w_all[:, idx * P:(idx + 1) * P] = lhst[(l, side, q)]
    w_all = w_all.astype(NPBF)

    in_maps = []
    for c in range(NC):
        idx = percore[c]
        s = len(idx)
        pad = G * w0 - s
        xs = x[idx][:, :t_steps, :]                      # [s, t_steps, 6]
        if pad:
            xs = np.concatenate([xs, np.zeros((pad, t_steps, HP), np.float32)], 0)
        xg = np.empty((KP, xcols), np.float32)
        xg[P, :] = 1.0
        for f in range(t_steps):
            v = vf[f]
            blk = xs[:G * v, f, :].reshape(v, G, HP)     # [col, g, k]
            xg[0:P, xoff[f]:xoff[f] + v] = blk.transpose(1, 2, 0).reshape(P, v)
        # masks: +40 -> frozen (z=1,u=0), -1e30 -> active (no-op under max)
        mg = np.full((P, max(mcols, 1)), -1e30, np.float32)
        for tt in range(t_steps):
            q, m, mo = qs[tt], ms[tt], moff[tt]
            if m <= 0:
                continue
            cnt = int(cnts[c, tt])
            qc, kc = cnt // G, cnt % G
            for j in range(q, wts[tt]):
                col = mg[:, mo + (j - q)]
                if j > qc or (j == qc and kc == 0):
                    col[:] = 40.0
                elif j == qc:
                    col[kc * HP:] = 40.0
        in_maps.append({"x_sb": xg.astype(NPBF), "w_all": w_all, "mask_t": mg,
                        "ones_t": np.ones((1, w0), NPBF)})

    nc = _build_program(t_steps, w0, wts, vf, xoff, xcols, qs, ms, moff, mcols,
                        n_dma_slices=10)
    bkr = run_bass_kernel_spmd(nc, in_maps, list(range(NC)), trace=trace)
    res = bkr.results

    out = np.zeros((n, HP), np.float32)
    for c in range(NC):
        idx = percore[c]
        og = np.asarray(res[c]["out"], dtype=np.float32)  # [P, w0]
        vals = og.reshape(G, HP, w0).transpose(2, 0, 1).reshape(G * w0, HP)
        out[idx] = vals[:len(idx)]
    return out, bkr
